# revision 53
# baseline (speedup 1.0000x reference)
"""ATMixer Trainium2 kernel (8-core data-parallel over batch).

Channel-major layout [C, N] per batch row:
 - deformable bilinear gather along N -> shifted free-dim APs + small window,
 - all big matmuls contract over channel partitions (f32r / bf16),
 - LayerNorm stats via PE ones-matmuls.

Exact 5-point bilinear form (offsets verified in (-2, 2)):
  sampled = x_n + A.Dp_n + B.Dp_{n-1} + C.Dp_{n+1} + D.Dp_{n-2}
  A=clamp(t,0,1) B=clamp(t,-1,0) C=relu(t-1) D=min(t+1,0),  Dp_j = x_{j+1}-x_j
"""

from contextlib import ExitStack

import numpy as np
import ml_dtypes

import concourse.bass as bass
import concourse.bacc as bacc
import concourse.tile as tile
import concourse.mybir as mybir
from concourse.bass_utils import run_bass_kernel_spmd

dt = mybir.dt
AF = mybir.ActivationFunctionType
OP = mybir.AluOpType

B, N, C, H = 16, 4096, 256, 64
NCORES = 8
BLOC = B // NCORES
PAD = 2
NP = N + 2 * PAD
CH = 1024                   # PSUM chunk along N
NCH = N // CH
EPS = 1e-6
P = 128
bf16 = ml_dtypes.bfloat16

_cache = {}


def _build_program(stop_after=""):
    import os
    stop_after = stop_after or os.environ.get("K_STOP", "")
    nc = bacc.Bacc("TRN2", target_bir_lowering=False)

    xb_d = nc.declare_dram_parameter("xb", [BLOC, C, NP], dt.bfloat16, isOutput=False)
    w_off_d = nc.declare_dram_parameter("w_off", [C, C], dt.bfloat16, isOutput=False)
    ncs_d = nc.declare_dram_parameter("neg_colsum", [1, C], dt.bfloat16, isOutput=False)
    w_n_d = nc.declare_dram_parameter("w_n", [C, C], dt.bfloat16, isOutput=False)
    w_atmc_d = nc.declare_dram_parameter("w_atmc", [C, C], dt.bfloat16, isOutput=False)
    w_atmc_t_d = nc.declare_dram_parameter("w_atmc_t", [C, C], dt.bfloat16, isOutput=False)
    w_proj_d = nc.declare_dram_parameter("w_proj", [C, C], dt.float32, isOutput=False)
    b_proj_d = nc.declare_dram_parameter("b_proj", [C, 1], dt.float32, isOutput=False)
    b_n_d = nc.declare_dram_parameter("b_n", [C, 1], dt.float32, isOutput=False)
    b_n_h_d = nc.declare_dram_parameter("b_n_h", [C, 1], dt.bfloat16, isOutput=False)
    w_fc1_d = nc.declare_dram_parameter("w_fc1", [C, H], dt.bfloat16, isOutput=False)
    b_fc1_d = nc.declare_dram_parameter("b_fc1", [H, 1], dt.float32, isOutput=False)
    b_fc1s_d = nc.declare_dram_parameter("b_fc1s", [H, 1], dt.float32, isOutput=False)
    w_fc2d_d = nc.declare_dram_parameter("w_fc2d", [H, C], dt.bfloat16, isOutput=False)
    b_fc2d_d = nc.declare_dram_parameter("b_fc2d", [C, 1], dt.float32, isOutput=False)
    out_d = nc.declare_dram_parameter("out_t", [BLOC, C, N], dt.float32, isOutput=True)

    f32r = dt.float32r

    with tile.TileContext(nc) as tc, ExitStack() as ctx:
            ep = ctx.enter_context
            const_pool = ep(tc.tile_pool(name="const", bufs=1))
            wts_pool = ep(tc.tile_pool(name="wts", bufs=1))
            xb_pool = ep(tc.tile_pool(name="xb", bufs=4))
            dp_pool = ep(tc.tile_pool(name="dp", bufs=2))
            t_pool = ep(tc.tile_pool(name="tt", bufs=2))
            scr_pool = ep(tc.tile_pool(name="scr", bufs=4))
            acc_pool = ep(tc.tile_pool(name="acc", bufs=3))
            smp_pool = ep(tc.tile_pool(name="smp", bufs=2))
            s_pool = ep(tc.tile_pool(name="s", bufs=2))
            rows_pool = ep(tc.tile_pool(name="rows", bufs=1))
            bcast_pool = ep(tc.tile_pool(name="bcast", bufs=1))
            osb_pool = ep(tc.tile_pool(name="osb", bufs=2))
            small_pool = ep(tc.tile_pool(name="small", bufs=1))
            ps_q = ep(tc.tile_pool(name="ps_q", bufs=2, space="PSUM"))
            ps_s = ep(tc.tile_pool(name="ps_s", bufs=2, space="PSUM"))
            ones_b = const_pool.tile([P, 1], dt.bfloat16, tag="c2")
            nc.vector.memset(ones_b[:], 1.0 / C)
            eps_col = const_pool.tile([P, 1], dt.float32, tag="c3")
            nc.vector.memset(eps_col[:], EPS)

            w_off_sb, w_n_sb, w_atmc_sb, w_atmc_t_sb, w_proj_sb = [], [], [], [], []
            w_fc1_sb, b_n_sb, b_n_h_sb, b_proj_sb, b_fc2d_sb = [], [], [], [], []
            for ct in range(2):
                sl = slice(ct * P, (ct + 1) * P)
                w = wts_pool.tile([P, C], dt.bfloat16, tag=f"w_off{ct}")
                nc.sync.dma_start(w[:], w_off_d[sl, :]); w_off_sb.append(w)
                w = wts_pool.tile([P, C], dt.bfloat16, tag=f"w_n{ct}")
                nc.sync.dma_start(w[:], w_n_d[sl, :]); w_n_sb.append(w)
                w = wts_pool.tile([P, C], dt.bfloat16, tag=f"w_atmc{ct}")
                nc.sync.dma_start(w[:], w_atmc_d[sl, :]); w_atmc_sb.append(w)
                w = wts_pool.tile([P, C], dt.bfloat16, tag=f"w_atmct{ct}")
                nc.sync.dma_start(w[:], w_atmc_t_d[sl, :]); w_atmc_t_sb.append(w)
                w = wts_pool.tile([P, C], dt.float32, tag=f"w_proj{ct}")
                nc.sync.dma_start(w[:], w_proj_d[sl, :]); w_proj_sb.append(w)
                w = wts_pool.tile([P, H], dt.bfloat16, tag=f"fc1_{ct}")
                nc.sync.dma_start(w[:], w_fc1_d[sl, :]); w_fc1_sb.append(w)
                w = wts_pool.tile([P, 1], dt.float32, tag=f"bn{ct}")
                nc.sync.dma_start(w[:], b_n_d[sl, :]); b_n_sb.append(w)
                w = wts_pool.tile([P, 1], dt.bfloat16, tag=f"bnh{ct}", name="bnh")
                nc.sync.dma_start(w[:], b_n_h_d[sl, :]); b_n_h_sb.append(w)
                w = wts_pool.tile([P, 1], dt.float32, tag=f"bp{ct}")
                nc.sync.dma_start(w[:], b_proj_d[sl, :]); b_proj_sb.append(w)
                w = wts_pool.tile([P, 1], dt.float32, tag=f"bf2{ct}")
                nc.sync.dma_start(w[:], b_fc2d_d[sl, :]); b_fc2d_sb.append(w)
            ncs_sb = wts_pool.tile([1, C], dt.bfloat16, tag="ncs")
            nc.sync.dma_start(ncs_sb[:], ncs_d[:, :])
            b_fc1_sb = wts_pool.tile([H, 1], dt.float32, tag="bfc1")
            nc.sync.dma_start(b_fc1_sb[:], b_fc1_d[:, :])
            b_fc1s_sb = wts_pool.tile([H, 1], dt.float32, tag="bfc1s")
            nc.sync.dma_start(b_fc1s_sb[:], b_fc1s_d[:, :])
            w_fc2d_sb = wts_pool.tile([H, C], dt.bfloat16, tag="fc2d")
            nc.sync.dma_start(w_fc2d_sb[:], w_fc2d_d[:, :])

            for b in range(BLOC):
                # ---- load + per-channel x sums ----
                xb_sb, xpool = [], []
                for ct in range(2):
                    xb_t = xb_pool.tile([P, NP], dt.bfloat16, tag="xb")
                    nc.sync.dma_start(xb_t[:], xb_d[b, ct * P:(ct + 1) * P, :])
                    xp = small_pool.tile([P, 1], dt.float32, tag=f"xp{ct}")
                    dead = scr_pool.tile([P, NP], dt.bfloat16, tag="sc", name="dead")
                    nc.scalar.activation(dead[:], xb_t[:], AF.Copy, accum_out=xp[:])
                    xb_sb.append(xb_t); xpool.append(xp)

                # ---- stats rows ----
                mu_h = rows_pool.tile([1, N], dt.bfloat16, tag="muh")
                istd_bc = bcast_pool.tile([P, N], dt.bfloat16, tag="ibc")
                xsq_sb = []
                for ct in range(2):
                    xsq_t = scr_pool.tile([P, N], dt.bfloat16, tag="sc")
                    nc.scalar.activation(xsq_t[:], xb_sb[ct][:, PAD:PAD + N], AF.Square)
                    xsq_sb.append(xsq_t)
                for q in range(NCH):
                    mu_ps = ps_q.tile([P, CH], dt.float32, tag="q")
                    ex_ps = ps_q.tile([P, CH], dt.float32, tag="q")
                    for j in range(CH // 512):
                        n0 = q * CH + j * 512
                        psl = slice(j * 512, (j + 1) * 512)
                        for ct in range(2):
                            nc.tensor.matmul(
                                mu_ps[0:1, psl], ones_b[:],
                                xb_sb[ct][:, PAD + n0:PAD + n0 + 512],
                                start=(ct == 0), stop=(ct == 1))
                            nc.tensor.matmul(
                                ex_ps[0:1, psl], ones_b[:], xsq_sb[ct][:, n0:n0 + 512],
                                start=(ct == 0), stop=(ct == 1))
                    nc.scalar.activation(mu_h[0:1, q * CH:(q + 1) * CH], mu_ps[0:1, :], AF.Copy)
                    musq_q = rows_pool.tile([1, CH], dt.bfloat16, tag="musq", bufs=2)
                    nc.vector.tensor_mul(musq_q[:], mu_h[0:1, q * CH:(q + 1) * CH],
                                         mu_h[0:1, q * CH:(q + 1) * CH])
                    var_q = rows_pool.tile([1, CH], dt.float32, tag="var", bufs=2)
                    nc.vector.scalar_tensor_tensor(var_q[:], musq_q[:], -1.0,
                                                   ex_ps[0:1, :], OP.mult, OP.add)
                    sq_q = rows_pool.tile([1, CH], dt.float32, tag="sq", bufs=1)
                    nc.scalar.activation(sq_q[:], var_q[:], AF.Sqrt, bias=eps_col[0:1, :])
                    istd_q = rows_pool.tile([1, CH], dt.float32, tag="istd", bufs=1)
                    nc.vector.reciprocal_approx_fast(istd_q[:], sq_q[:])
                    istd_h = rows_pool.tile([1, CH], dt.bfloat16, tag="istdh", bufs=2)
                    nc.vector.tensor_copy(istd_h[:], istd_q[:])
                    nc.gpsimd.partition_broadcast(istd_bc[:, q * CH:(q + 1) * CH],
                                                  istd_h[0:1, :], channels=P)

                if stop_after == "stats":
                    continue
                # ---- per channel-half: offsets Q -> t -> interp ----
                smp_sb = []
                for ot in range(2):
                    t_t = t_pool.tile([P, N], dt.bfloat16, tag="t")
                    for q in range(NCH):
                        q_ps = ps_q.tile([P, CH], dt.float32, tag="q")
                        for j in range(CH // 512):
                            n0 = q * CH + j * 512
                            psl = slice(j * 512, (j + 1) * 512)
                            for ct in range(2):
                                nc.tensor.matmul(
                                    q_ps[:, psl],
                                    w_off_sb[ct][:, ot * P:(ot + 1) * P],
                                    xb_sb[ct][:, PAD + n0:PAD + n0 + 512],
                                    start=(ct == 0), stop=False)
                            nc.tensor.matmul(
                                q_ps[:, psl],
                                ncs_sb[0:1, ot * P:(ot + 1) * P],
                                mu_h[0:1, n0:n0 + 512],
                                start=False, stop=True)
                        nc.vector.scalar_tensor_tensor(
                            t_t[:, q * CH:(q + 1) * CH], q_ps[:], 1.0,
                            istd_bc[:, q * CH:(q + 1) * CH], OP.mult, OP.mult)
                    dp = dp_pool.tile([P, NP - 1], dt.bfloat16, tag="dp")
                    nc.vector.tensor_sub(dp[:], xb_sb[ot][:, 1:NP], xb_sb[ot][:, 0:NP - 1])
                    smp_halves = [smp_pool.tile([P, N // 2], dt.bfloat16, tag="smp",
                                                bufs=4, name=f"smp{ot}_{h}")
                                  for h in range(2)]
                    tA = scr_pool.tile([P, N], dt.bfloat16, tag="sc")
                    tB = scr_pool.tile([P, N], dt.bfloat16, tag="sc")
                    tC = scr_pool.tile([P, N], dt.bfloat16, tag="sc")
                    tD = scr_pool.tile([P, N], dt.bfloat16, tag="sc")
                    nc.vector.tensor_scalar(tA[:], t_t[:], 0.0, 1.0, OP.max, OP.min)
                    nc.vector.tensor_scalar(tB[:], t_t[:], 0.0, -1.0, OP.min, OP.max)
                    nc.vector.tensor_scalar(tC[:], t_t[:], 1.0, 0.0, OP.subtract, OP.max)
                    nc.vector.tensor_scalar(tD[:], t_t[:], 1.0, 0.0, OP.add, OP.min)
                    for hn in range(2):
                        HW = N // 2
                        sl = slice(hn * HW, (hn + 1) * HW)
                        j0 = hn * HW + PAD     # halo coord of first n in slice
                        p1 = scr_pool.tile([P, HW], dt.bfloat16, tag="pc", bufs=2)
                        a1 = acc_pool.tile([P, HW], dt.bfloat16, tag="ac")
                        nc.vector.tensor_mul(p1[:], tA[:, sl], dp[:, j0:j0 + HW])
                        nc.vector.tensor_add(a1[:], xb_sb[ot][:, j0:j0 + HW], p1[:])
                        p2 = scr_pool.tile([P, HW], dt.bfloat16, tag="pc", bufs=2)
                        a2 = acc_pool.tile([P, HW], dt.bfloat16, tag="ac")
                        nc.vector.tensor_mul(p2[:], tB[:, sl], dp[:, j0 - 1:j0 - 1 + HW])
                        nc.vector.tensor_add(a2[:], a1[:], p2[:])
                        p3 = scr_pool.tile([P, HW], dt.bfloat16, tag="pc", bufs=2)
                        a3 = acc_pool.tile([P, HW], dt.bfloat16, tag="ac")
                        nc.vector.tensor_mul(p3[:], tC[:, sl], dp[:, j0 + 1:j0 + 1 + HW])
                        nc.vector.tensor_add(a3[:], a2[:], p3[:])
                        p4 = scr_pool.tile([P, HW], dt.bfloat16, tag="pc", bufs=2)
                        nc.vector.tensor_mul(p4[:], tD[:, sl], dp[:, j0 - 2:j0 - 2 + HW])
                        nc.vector.tensor_add(smp_halves[hn][:], a3[:], p4[:])
                    smp_sb.append(smp_halves)

                if stop_after == "interp":
                    continue
                # ---- s = atm_n(sampled); evac + pool (all-ones mask) ----
                ps_acc = [small_pool.tile([P, NCH], dt.float32, tag=f"pa{o}",
                                          name=f"psacc{o}") for o in range(2)]
                s_sb = [[s_pool.tile([P, N // 2], dt.bfloat16, tag="s", bufs=4,
                                     name=f"s{o}_{h}") for h in range(2)]
                        for o in range(2)]
                for q in range(NCH):
                    hf, qr = q // 2, q % 2
                    for ot in range(2):
                        s_ps = ps_s.tile([P, CH], dt.float32, tag="s")
                        for j in range(CH // 512):
                            n0 = qr * CH + j * 512
                            psl = slice(j * 512, (j + 1) * 512)
                            for ct in range(2):
                                nc.tensor.matmul(
                                    s_ps[:, psl],
                                    w_n_sb[ct][:, ot * P:(ot + 1) * P],
                                    smp_sb[ct][hf][:, n0:n0 + 512],
                                    start=(ct == 0), stop=(ct == 1))
                        nc.scalar.activation(
                            s_sb[ot][hf][:, qr * CH:(qr + 1) * CH], s_ps[:], AF.Copy,
                            accum_out=ps_acc[ot][:, q:q + 1])

                if stop_after == "atmn":
                    continue
                # ---- pools + gating ----
                a_col = []
                for ot in range(2):
                    pool_s = small_pool.tile([P, 1], dt.float32, tag=f"ps{ot}")
                    nc.vector.tensor_reduce(pool_s[:], ps_acc[ot][:],
                                            mybir.AxisListType.X, OP.add)
                    xpb = small_pool.tile([P, 1], dt.bfloat16, tag=f"xpb{ot}")
                    nc.vector.tensor_copy(xpb[:], xpool[ot][:])
                    a_col.append((pool_s, xpb))
                xpb_all = [a_col[0][1], a_col[1][1]]
                a_cols = []
                for ot in range(2):
                    pc_ps = ps_q.tile([P, 1], dt.float32, tag="q")
                    for ct in range(2):
                        nc.tensor.matmul(pc_ps[:], w_atmc_sb[ct][:, ot * P:(ot + 1) * P],
                                         xpb_all[ct][:], start=(ct == 0), stop=(ct == 1))
                    apre = small_pool.tile([P, 1], dt.float32, tag=f"apre{ot}")
                    nc.vector.scalar_tensor_tensor(apre[:], pc_ps[:], 1.0,
                                                   a_col[ot][0][:], OP.mult, OP.add)
                    ac = small_pool.tile([P, 1], dt.bfloat16, tag=f"acol{ot}")
                    nc.vector.tensor_scalar(ac[:], apre[:], 1.0 / N, b_n_sb[ot][:],
                                            OP.mult, OP.add)
                    a_cols.append(ac)
                h_ps = ps_q.tile([H, 1], dt.float32, tag="q")
                for ct in range(2):
                    nc.tensor.matmul(h_ps[:], w_fc1_sb[ct][:], a_cols[ct][:],
                                     start=(ct == 0), stop=(ct == 1))
                pre_col = small_pool.tile([H, 1], dt.float32, tag="prec")
                nc.scalar.activation(pre_col[:], h_ps[:], AF.Identity, bias=b_fc1_sb[:])
                erf_col = small_pool.tile([H, 1], dt.float32, tag="erfc")
                nc.scalar.activation(erf_col[:], h_ps[:], AF.Erf, bias=b_fc1s_sb[:],
                                     scale=0.7071067811865476)
                hg_pre = small_pool.tile([H, 1], dt.float32, tag="hgp")
                nc.vector.scalar_tensor_tensor(hg_pre[:], erf_col[:], 1.0, pre_col[:],
                                               OP.add, OP.mult)
                hg = small_pool.tile([H, 1], dt.bfloat16, tag="hg")
                nc.vector.tensor_scalar_mul(hg[:], hg_pre[:], 0.5)
                g0, g1 = [], []
                for ot in range(2):
                    z_ps = ps_q.tile([P, 1], dt.float32, tag="q")
                    nc.tensor.matmul(z_ps[:], w_fc2d_sb[:, ot * P:(ot + 1) * P], hg[:],
                                     start=True, stop=True)
                    g0t = small_pool.tile([P, 1], dt.float32, tag=f"g0{ot}")
                    nc.scalar.activation(g0t[:], z_ps[:], AF.Sigmoid, bias=b_fc2d_sb[ot][:])
                    g1t = small_pool.tile([P, 1], dt.float32, tag=f"g1{ot}")
                    nc.vector.tensor_scalar(g1t[:], g0t[:], -1.0, 1.0, OP.mult, OP.add)
                    g0.append(g0t); g1.append(g1t)

                ws_sb, w1_sb = [], []
                for ct in range(2):
                    ws = small_pool.tile([P, C], dt.bfloat16, tag=f"ws{ct}")
                    nc.vector.tensor_scalar_mul(ws[:], w_proj_sb[ct][:], g0[ct][:])
                    ws_sb.append(ws)
                    w1 = small_pool.tile([P, C], dt.bfloat16, tag=f"w1{ct}")
                    nc.vector.tensor_scalar_mul(w1[:], w_proj_sb[ct][:], g1[ct][:])
                    w1_sb.append(w1)
                wcp_sb = []
                for ci in range(2):
                    wcp_ps = ps_q.tile([P, C], dt.float32, tag="q")
                    for mt in range(2):
                        nc.tensor.matmul(wcp_ps[:], w_atmc_t_sb[mt][:, ci * P:(ci + 1) * P],
                                         w1_sb[mt][:], start=(mt == 0), stop=(mt == 1))
                    wcp = small_pool.tile([P, C], dt.bfloat16, tag=f"wcp{ci}")
                    nc.vector.tensor_copy(wcp[:], wcp_ps[:])
                    wcp_sb.append(wcp)
                btot = []
                for ot in range(2):
                    vv_ps = ps_q.tile([P, 1], dt.float32, tag="q")
                    for ct in range(2):
                        nc.tensor.matmul(vv_ps[:], ws_sb[ct][:, ot * P:(ot + 1) * P],
                                         b_n_h_sb[ct][:], start=(ct == 0), stop=(ct == 1))
                    bt = small_pool.tile([P, 1], dt.float32, tag=f"bt{ot}")
                    nc.vector.scalar_tensor_tensor(bt[:], vv_ps[:], 1.0,
                                                   b_proj_sb[ot][:], OP.mult, OP.add)
                    btot.append(bt)

                if stop_after == "gating":
                    continue
                # ---- final matmuls + bias + store ----
                for ot in range(2):
                    for q in range(NCH):
                        o_ps = ps_s.tile([P, CH], dt.float32, tag="s")
                        for j in range(CH // 512):
                            n0 = q * CH + j * 512
                            psl = slice(j * 512, (j + 1) * 512)
                            for ct in range(2):
                                nc.tensor.matmul(
                                    o_ps[:, psl], ws_sb[ct][:, ot * P:(ot + 1) * P],
                                    s_sb[ct][q // 2][:, (q % 2) * CH + j * 512:
                                                     (q % 2) * CH + (j + 1) * 512],
                                    start=(ct == 0), stop=False)
                            for ci in range(2):
                                nc.tensor.matmul(
                                    o_ps[:, psl], wcp_sb[ci][:, ot * P:(ot + 1) * P],
                                    xb_sb[ci][:, PAD + n0:PAD + n0 + 512],
                                    start=False, stop=(ci == 1))
                        o_sb = osb_pool.tile([P, CH], dt.float32, tag="osb")
                        nc.scalar.activation(o_sb[:], o_ps[:], AF.Identity, bias=btot[ot][:])
                        nc.sync.dma_start(
                            out_d[b, ot * P:(ot + 1) * P, q * CH:(q + 1) * CH], o_sb[:])
    nc.compile()
    return nc


def _host_prep(inputs):
    x = inputs["x"].astype(np.float32)
    ln_g, ln_b = inputs["ln_g"], inputs["ln_b"]
    off_w, off_b = inputs["off_w"], inputs["off_b"]
    w_off = (ln_g[:, None] * off_w).astype(np.float32)
    off_b_eff = ln_b @ off_w + off_b
    assert np.allclose(off_b_eff, 0.0), "offset bias path not implemented"
    neg_colsum = (-w_off.sum(0, dtype=np.float64)).astype(np.float32)[None, :]
    fc2_w, fc2_b = inputs["fc2_w"], inputs["fc2_b"]
    fc2d_w = (fc2_w[:, 0::2] - fc2_w[:, 1::2]).astype(np.float32)
    fc2d_b = (fc2_b[0::2] - fc2_b[1::2]).astype(np.float32)
    xt = np.ascontiguousarray(x.transpose(0, 2, 1))
    xt = np.pad(xt, ((0, 0), (0, 0), (PAD, PAD))).astype(bf16)
    common = {
        "w_off": w_off.astype(bf16),
        "neg_colsum": neg_colsum.astype(bf16),
        "w_n": inputs["atm_n_w"].astype(bf16),
        "w_atmc": inputs["atm_c_w"].astype(bf16),
        "w_atmc_t": np.ascontiguousarray(inputs["atm_c_w"].T).astype(bf16),
        "w_proj": inputs["proj_w"].astype(np.float32),
        "b_proj": inputs["proj_b"].astype(np.float32)[:, None],
        "b_n": inputs["atm_n_b"].astype(np.float32)[:, None],
        "b_n_h": inputs["atm_n_b"].astype(bf16)[:, None],
        "w_fc1": inputs["fc1_w"].astype(bf16),
        "b_fc1": inputs["fc1_b"].astype(np.float32)[:, None],
        "b_fc1s": (inputs["fc1_b"] * 0.7071067811865476).astype(np.float32)[:, None],
        "w_fc2d": fc2d_w.astype(bf16),
        "b_fc2d": fc2d_b[:, None],
    }
    in_maps = []
    for core in range(NCORES):
        m = dict(common)
        m["xb"] = np.ascontiguousarray(xt[core * BLOC:(core + 1) * BLOC])
        in_maps.append(m)
    return in_maps


def kernel(**inputs):
    inputs = {k: np.asarray(v) for k, v in inputs.items()}
    assert np.all(inputs["attn_mask"] == 1), "kernel specialized for all-ones mask"
    in_maps = _host_prep(inputs)
    if "nc" not in _cache:
        _cache["nc"] = _build_program()
    res = run_bass_kernel_spmd(_cache["nc"], in_maps, list(range(NCORES)))
    outs = [res.results[i]["out_t"] for i in range(NCORES)]
    full = np.concatenate(outs, axis=0)
    return np.ascontiguousarray(full.transpose(0, 2, 1))


if __name__ == "__main__":
    import reference
    inp = {k: np.asarray(v) for k, v in reference.setup_inputs().items()}
    out = kernel(**inp)
    print("kernel out", out.shape, out.dtype)


# revision 61
# speedup vs baseline: 1.0583x; 1.0583x over previous
"""ATMixer Trainium2 kernel (8-core data-parallel over batch).

Channel-major layout [C, N] per batch row:
 - deformable bilinear gather along N -> shifted free-dim APs + small window,
 - all big matmuls contract over channel partitions (f32r / bf16),
 - LayerNorm stats via PE ones-matmuls.

Exact 5-point bilinear form (offsets verified in (-2, 2)):
  sampled = x_n + A.Dp_n + B.Dp_{n-1} + C.Dp_{n+1} + D.Dp_{n-2}
  A=clamp(t,0,1) B=clamp(t,-1,0) C=relu(t-1) D=min(t+1,0),  Dp_j = x_{j+1}-x_j
"""

from contextlib import ExitStack

import numpy as np
import ml_dtypes

import concourse.bass as bass
import concourse.bacc as bacc
import concourse.tile as tile
import concourse.mybir as mybir
from concourse.bass_utils import run_bass_kernel_spmd

dt = mybir.dt
AF = mybir.ActivationFunctionType
OP = mybir.AluOpType

B, N, C, H = 16, 4096, 256, 64
NCORES = 8
BLOC = B // NCORES
PAD = 2
NP = N + 2 * PAD
CH = 1024                   # PSUM chunk along N
NCH = N // CH
EPS = 1e-6
P = 128
bf16 = ml_dtypes.bfloat16

_cache = {}


def _build_program(stop_after=""):
    import os
    stop_after = stop_after or os.environ.get("K_STOP", "")
    nc = bacc.Bacc("TRN2", target_bir_lowering=False)

    xb_d = nc.declare_dram_parameter("xb", [BLOC, C, NP], dt.bfloat16, isOutput=False)
    w_off_d = nc.declare_dram_parameter("w_off", [C, C], dt.bfloat16, isOutput=False)
    ncs_d = nc.declare_dram_parameter("neg_colsum", [1, C], dt.bfloat16, isOutput=False)
    w_n_d = nc.declare_dram_parameter("w_n", [C, C], dt.bfloat16, isOutput=False)
    w_atmc_d = nc.declare_dram_parameter("w_atmc", [C, C], dt.bfloat16, isOutput=False)
    w_atmc_t_d = nc.declare_dram_parameter("w_atmc_t", [C, C], dt.bfloat16, isOutput=False)
    w_proj_d = nc.declare_dram_parameter("w_proj", [C, C], dt.float32, isOutput=False)
    b_proj_d = nc.declare_dram_parameter("b_proj", [C, 1], dt.float32, isOutput=False)
    b_n_d = nc.declare_dram_parameter("b_n", [C, 1], dt.float32, isOutput=False)
    b_n_h_d = nc.declare_dram_parameter("b_n_h", [C, 1], dt.bfloat16, isOutput=False)
    w_fc1_d = nc.declare_dram_parameter("w_fc1", [C, H], dt.bfloat16, isOutput=False)
    b_fc1_d = nc.declare_dram_parameter("b_fc1", [H, 1], dt.float32, isOutput=False)
    b_fc1s_d = nc.declare_dram_parameter("b_fc1s", [H, 1], dt.float32, isOutput=False)
    w_fc2d_d = nc.declare_dram_parameter("w_fc2d", [H, C], dt.bfloat16, isOutput=False)
    b_fc2d_d = nc.declare_dram_parameter("b_fc2d", [C, 1], dt.float32, isOutput=False)
    out_d = nc.declare_dram_parameter("out_t", [BLOC, C, N], dt.float32, isOutput=True)

    f32r = dt.float32r

    with tile.TileContext(nc) as tc, ExitStack() as ctx:
            ep = ctx.enter_context
            const_pool = ep(tc.tile_pool(name="const", bufs=1))
            wts_pool = ep(tc.tile_pool(name="wts", bufs=1))
            xb_pool = ep(tc.tile_pool(name="xb", bufs=4))
            dp_pool = ep(tc.tile_pool(name="dp", bufs=2))
            t_pool = ep(tc.tile_pool(name="tt", bufs=2))
            scr_pool = ep(tc.tile_pool(name="scr", bufs=3))
            acc_pool = ep(tc.tile_pool(name="acc", bufs=2))
            smp_pool = ep(tc.tile_pool(name="smp", bufs=2))
            s_pool = ep(tc.tile_pool(name="s", bufs=2))
            rows_pool = ep(tc.tile_pool(name="rows", bufs=1))
            bcast_pool = ep(tc.tile_pool(name="bcast", bufs=1))
            osb_pool = ep(tc.tile_pool(name="osb", bufs=2))
            small_pool = ep(tc.tile_pool(name="small", bufs=1))
            ps_q = ep(tc.tile_pool(name="ps_q", bufs=2, space="PSUM"))
            ps_s = ep(tc.tile_pool(name="ps_s", bufs=2, space="PSUM"))
            ones_b = const_pool.tile([P, 1], dt.bfloat16, tag="c2")
            nc.vector.memset(ones_b[:], 1.0 / C)
            eps_col = const_pool.tile([1, 1], dt.float32, tag="c3")
            nc.vector.memset(eps_col[:], EPS)

            w_off_sb, w_n_sb, w_atmc_sb, w_atmc_t_sb, w_proj_sb = [], [], [], [], []
            w_fc1_sb, b_n_sb, b_n_h_sb, b_proj_sb, b_fc2d_sb = [], [], [], [], []
            for ct in range(2):
                sl = slice(ct * P, (ct + 1) * P)
                w = wts_pool.tile([P, C], dt.bfloat16, tag=f"w_off{ct}")
                nc.sync.dma_start(w[:], w_off_d[sl, :]); w_off_sb.append(w)
                w = wts_pool.tile([P, C], dt.bfloat16, tag=f"w_n{ct}")
                nc.sync.dma_start(w[:], w_n_d[sl, :]); w_n_sb.append(w)
                w = wts_pool.tile([P, C], dt.bfloat16, tag=f"w_atmc{ct}")
                nc.sync.dma_start(w[:], w_atmc_d[sl, :]); w_atmc_sb.append(w)
                w = wts_pool.tile([P, C], dt.bfloat16, tag=f"w_atmct{ct}")
                nc.sync.dma_start(w[:], w_atmc_t_d[sl, :]); w_atmc_t_sb.append(w)
                w = wts_pool.tile([P, C], dt.float32, tag=f"w_proj{ct}")
                nc.sync.dma_start(w[:], w_proj_d[sl, :]); w_proj_sb.append(w)
                w = wts_pool.tile([P, H], dt.bfloat16, tag=f"fc1_{ct}")
                nc.sync.dma_start(w[:], w_fc1_d[sl, :]); w_fc1_sb.append(w)
                w = wts_pool.tile([P, 1], dt.float32, tag=f"bn{ct}")
                nc.sync.dma_start(w[:], b_n_d[sl, :]); b_n_sb.append(w)
                w = wts_pool.tile([P, 1], dt.bfloat16, tag=f"bnh{ct}", name="bnh")
                nc.sync.dma_start(w[:], b_n_h_d[sl, :]); b_n_h_sb.append(w)
                w = wts_pool.tile([P, 1], dt.float32, tag=f"bp{ct}")
                nc.sync.dma_start(w[:], b_proj_d[sl, :]); b_proj_sb.append(w)
                w = wts_pool.tile([P, 1], dt.float32, tag=f"bf2{ct}")
                nc.sync.dma_start(w[:], b_fc2d_d[sl, :]); b_fc2d_sb.append(w)
            ncs_sb = wts_pool.tile([1, C], dt.bfloat16, tag="ncs")
            nc.sync.dma_start(ncs_sb[:], ncs_d[:, :])
            b_fc1_sb = wts_pool.tile([H, 1], dt.float32, tag="bfc1")
            nc.sync.dma_start(b_fc1_sb[:], b_fc1_d[:, :])
            b_fc1s_sb = wts_pool.tile([H, 1], dt.float32, tag="bfc1s")
            nc.sync.dma_start(b_fc1s_sb[:], b_fc1s_d[:, :])
            w_fc2d_sb = wts_pool.tile([H, C], dt.bfloat16, tag="fc2d")
            nc.sync.dma_start(w_fc2d_sb[:], w_fc2d_d[:, :])

            for b in range(BLOC):
                # ---- load + per-channel x sums ----
                xb_sb, xpool = [], []
                for ct in range(2):
                    xb_t = xb_pool.tile([P, NP], dt.bfloat16, tag="xb")
                    nc.sync.dma_start(xb_t[:], xb_d[b, ct * P:(ct + 1) * P, :])
                    xb_sb.append(xb_t)

                # ---- stats rows (squares first: they gate stats -> Q) ----
                mu_h = rows_pool.tile([1, N], dt.bfloat16, tag="muh")
                istd_bc = bcast_pool.tile([P, N], dt.bfloat16, tag="ibc")
                xsq_sb = []
                for ct in range(2):
                    xsq_t = scr_pool.tile([P, N], dt.bfloat16, tag="sc")
                    nc.scalar.activation(xsq_t[:], xb_sb[ct][:, PAD:PAD + N], AF.Square)
                    xsq_sb.append(xsq_t)
                for ct in range(2):
                    xp = small_pool.tile([P, 1], dt.float32, tag=f"xp{ct}", name="xp")
                    dead = scr_pool.tile([P, NP], dt.bfloat16, tag="sc", name="dead")
                    nc.scalar.activation(dead[:], xb_sb[ct][:], AF.Copy, accum_out=xp[:])
                    xpool.append(xp)
                for q in range(NCH):
                    mu_ps = ps_q.tile([P, CH], dt.float32, tag="q")
                    ex_ps = ps_q.tile([P, CH], dt.float32, tag="q")
                    for j in range(CH // 512):
                        n0 = q * CH + j * 512
                        psl = slice(j * 512, (j + 1) * 512)
                        for ct in range(2):
                            nc.tensor.matmul(
                                mu_ps[0:1, psl], ones_b[:],
                                xb_sb[ct][:, PAD + n0:PAD + n0 + 512],
                                start=(ct == 0), stop=(ct == 1))
                    for j in range(CH // 512):
                        n0 = q * CH + j * 512
                        psl = slice(j * 512, (j + 1) * 512)
                        for ct in range(2):
                            nc.tensor.matmul(
                                ex_ps[0:1, psl], ones_b[:], xsq_sb[ct][:, n0:n0 + 512],
                                start=(ct == 0), stop=(ct == 1))
                    nc.scalar.activation(mu_h[0:1, q * CH:(q + 1) * CH], mu_ps[0:1, :], AF.Copy)
                    musq_q = rows_pool.tile([1, CH], dt.bfloat16, tag="musq", bufs=2)
                    nc.vector.tensor_mul(musq_q[:], mu_h[0:1, q * CH:(q + 1) * CH],
                                         mu_h[0:1, q * CH:(q + 1) * CH])
                    var_q = rows_pool.tile([1, CH], dt.float32, tag="var", bufs=2)
                    nc.vector.scalar_tensor_tensor(var_q[:], musq_q[:], -1.0,
                                                   ex_ps[0:1, :], OP.mult, OP.add)
                    sq_q = rows_pool.tile([1, CH], dt.float32, tag="sq", bufs=1)
                    nc.scalar.activation(sq_q[:], var_q[:], AF.Sqrt, bias=eps_col[:])
                    istd_q = rows_pool.tile([1, CH], dt.float32, tag="istd", bufs=1)
                    nc.vector.reciprocal_approx_fast(istd_q[:], sq_q[:])
                    istd_h = rows_pool.tile([1, CH], dt.bfloat16, tag="istdh", bufs=2)
                    nc.vector.tensor_copy(istd_h[:], istd_q[:])
                    nc.gpsimd.partition_broadcast(istd_bc[:, q * CH:(q + 1) * CH],
                                                  istd_h[0:1, :], channels=P)

                if stop_after == "stats":
                    continue
                # ---- per channel-half: offsets Q -> t -> interp ----
                smp_sb = []
                for ot in range(2):
                    t_t = t_pool.tile([P, N], dt.bfloat16, tag="t")
                    for q in range(NCH):
                        q_ps = ps_q.tile([P, CH], dt.float32, tag="q")
                        for j in range(CH // 512):
                            n0 = q * CH + j * 512
                            psl = slice(j * 512, (j + 1) * 512)
                            for ct in range(2):
                                nc.tensor.matmul(
                                    q_ps[:, psl],
                                    w_off_sb[ct][:, ot * P:(ot + 1) * P],
                                    xb_sb[ct][:, PAD + n0:PAD + n0 + 512],
                                    start=(ct == 0), stop=False)
                            nc.tensor.matmul(
                                q_ps[:, psl],
                                ncs_sb[0:1, ot * P:(ot + 1) * P],
                                mu_h[0:1, n0:n0 + 512],
                                start=False, stop=True)
                        nc.vector.scalar_tensor_tensor(
                            t_t[:, q * CH:(q + 1) * CH], q_ps[:], 1.0,
                            istd_bc[:, q * CH:(q + 1) * CH], OP.mult, OP.mult)
                    dp = dp_pool.tile([P, NP - 1], dt.bfloat16, tag="dp")
                    nc.vector.tensor_sub(dp[:], xb_sb[ot][:, 1:NP], xb_sb[ot][:, 0:NP - 1])
                    smp_halves = [smp_pool.tile([P, N // 2], dt.bfloat16, tag="smp",
                                                bufs=4, name=f"smp{ot}_{h}")
                                  for h in range(2)]
                    for hn in range(2):
                        HW = N // 2
                        sl = slice(hn * HW, (hn + 1) * HW)
                        j0 = hn * HW + PAD     # halo coord of first n in slice
                        tA = scr_pool.tile([P, HW], dt.bfloat16, tag="wt", bufs=3)
                        p1 = scr_pool.tile([P, HW], dt.bfloat16, tag="pc", bufs=2)
                        a1 = acc_pool.tile([P, HW], dt.bfloat16, tag="ac")
                        nc.vector.tensor_scalar(tA[:], t_t[:, sl], 0.0, 1.0, OP.max, OP.min)
                        nc.vector.tensor_mul(p1[:], tA[:], dp[:, j0:j0 + HW])
                        nc.vector.tensor_add(a1[:], xb_sb[ot][:, j0:j0 + HW], p1[:])
                        tB = scr_pool.tile([P, HW], dt.bfloat16, tag="wt", bufs=3)
                        p2 = scr_pool.tile([P, HW], dt.bfloat16, tag="pc", bufs=2)
                        a2 = acc_pool.tile([P, HW], dt.bfloat16, tag="ac")
                        nc.vector.tensor_scalar(tB[:], t_t[:, sl], 0.0, -1.0, OP.min, OP.max)
                        nc.vector.tensor_mul(p2[:], tB[:], dp[:, j0 - 1:j0 - 1 + HW])
                        nc.vector.tensor_add(a2[:], a1[:], p2[:])
                        tC = scr_pool.tile([P, HW], dt.bfloat16, tag="wt", bufs=3)
                        p3 = scr_pool.tile([P, HW], dt.bfloat16, tag="pc", bufs=2)
                        a3 = acc_pool.tile([P, HW], dt.bfloat16, tag="ac")
                        nc.vector.tensor_scalar(tC[:], t_t[:, sl], 1.0, 0.0, OP.subtract, OP.max)
                        nc.vector.tensor_mul(p3[:], tC[:], dp[:, j0 + 1:j0 + 1 + HW])
                        nc.vector.tensor_add(a3[:], a2[:], p3[:])
                        tD = scr_pool.tile([P, HW], dt.bfloat16, tag="wt", bufs=3)
                        p4 = scr_pool.tile([P, HW], dt.bfloat16, tag="pc", bufs=2)
                        nc.vector.tensor_scalar(tD[:], t_t[:, sl], 1.0, 0.0, OP.add, OP.min)
                        nc.vector.tensor_mul(p4[:], tD[:], dp[:, j0 - 2:j0 - 2 + HW])
                        nc.vector.tensor_add(smp_halves[hn][:], a3[:], p4[:])
                    smp_sb.append(smp_halves)

                if stop_after == "interp":
                    continue
                # ---- s = atm_n(sampled); evac + pool (all-ones mask) ----
                ps_acc = [small_pool.tile([P, NCH], dt.float32, tag=f"pa{o}",
                                          name=f"psacc{o}") for o in range(2)]
                s_sb = [[s_pool.tile([P, N // 2], dt.bfloat16, tag="s", bufs=4,
                                     name=f"s{o}_{h}") for h in range(2)]
                        for o in range(2)]
                for q in range(NCH):
                    hf, qr = q // 2, q % 2
                    for ot in range(2):
                        s_ps = ps_s.tile([P, CH], dt.float32, tag="s")
                        for j in range(CH // 512):
                            n0 = qr * CH + j * 512
                            psl = slice(j * 512, (j + 1) * 512)
                            for ct in range(2):
                                nc.tensor.matmul(
                                    s_ps[:, psl],
                                    w_n_sb[ct][:, ot * P:(ot + 1) * P],
                                    smp_sb[ct][hf][:, n0:n0 + 512],
                                    start=(ct == 0), stop=(ct == 1))
                        nc.scalar.activation(
                            s_sb[ot][hf][:, qr * CH:(qr + 1) * CH], s_ps[:], AF.Copy,
                            accum_out=ps_acc[ot][:, q:q + 1])

                if stop_after == "atmn":
                    continue
                # ---- pools + gating ----
                a_col = []
                for ot in range(2):
                    pool_s = small_pool.tile([P, 1], dt.float32, tag=f"ps{ot}")
                    nc.vector.tensor_reduce(pool_s[:], ps_acc[ot][:],
                                            mybir.AxisListType.X, OP.add)
                    xpb = small_pool.tile([P, 1], dt.bfloat16, tag=f"xpb{ot}")
                    nc.vector.tensor_copy(xpb[:], xpool[ot][:])
                    a_col.append((pool_s, xpb))
                xpb_all = [a_col[0][1], a_col[1][1]]
                a_cols = []
                for ot in range(2):
                    pc_ps = ps_q.tile([P, 1], dt.float32, tag="q")
                    for ct in range(2):
                        nc.tensor.matmul(pc_ps[:], w_atmc_sb[ct][:, ot * P:(ot + 1) * P],
                                         xpb_all[ct][:], start=(ct == 0), stop=(ct == 1))
                    apre = small_pool.tile([P, 1], dt.float32, tag=f"apre{ot}")
                    nc.vector.scalar_tensor_tensor(apre[:], pc_ps[:], 1.0,
                                                   a_col[ot][0][:], OP.mult, OP.add)
                    ac = small_pool.tile([P, 1], dt.bfloat16, tag=f"acol{ot}")
                    nc.vector.tensor_scalar(ac[:], apre[:], 1.0 / N, b_n_sb[ot][:],
                                            OP.mult, OP.add)
                    a_cols.append(ac)
                h_ps = ps_q.tile([H, 1], dt.float32, tag="q")
                for ct in range(2):
                    nc.tensor.matmul(h_ps[:], w_fc1_sb[ct][:], a_cols[ct][:],
                                     start=(ct == 0), stop=(ct == 1))
                pre_col = small_pool.tile([H, 1], dt.float32, tag="prec")
                nc.scalar.activation(pre_col[:], h_ps[:], AF.Identity, bias=b_fc1_sb[:])
                erf_col = small_pool.tile([H, 1], dt.float32, tag="erfc")
                nc.scalar.activation(erf_col[:], h_ps[:], AF.Erf, bias=b_fc1s_sb[:],
                                     scale=0.7071067811865476)
                hg_pre = small_pool.tile([H, 1], dt.float32, tag="hgp")
                nc.vector.scalar_tensor_tensor(hg_pre[:], erf_col[:], 1.0, pre_col[:],
                                               OP.add, OP.mult)
                hg = small_pool.tile([H, 1], dt.bfloat16, tag="hg")
                nc.vector.tensor_scalar_mul(hg[:], hg_pre[:], 0.5)
                g0, g1 = [], []
                for ot in range(2):
                    z_ps = ps_q.tile([P, 1], dt.float32, tag="q")
                    nc.tensor.matmul(z_ps[:], w_fc2d_sb[:, ot * P:(ot + 1) * P], hg[:],
                                     start=True, stop=True)
                    g0t = small_pool.tile([P, 1], dt.float32, tag=f"g0{ot}")
                    nc.scalar.activation(g0t[:], z_ps[:], AF.Sigmoid, bias=b_fc2d_sb[ot][:])
                    g1t = small_pool.tile([P, 1], dt.float32, tag=f"g1{ot}")
                    nc.vector.tensor_scalar(g1t[:], g0t[:], -1.0, 1.0, OP.mult, OP.add)
                    g0.append(g0t); g1.append(g1t)

                ws_sb, w1_sb = [], []
                for ct in range(2):
                    ws = small_pool.tile([P, C], dt.bfloat16, tag=f"ws{ct}")
                    nc.vector.tensor_scalar_mul(ws[:], w_proj_sb[ct][:], g0[ct][:])
                    ws_sb.append(ws)
                    w1 = small_pool.tile([P, C], dt.bfloat16, tag=f"w1{ct}")
                    nc.vector.tensor_scalar_mul(w1[:], w_proj_sb[ct][:], g1[ct][:])
                    w1_sb.append(w1)
                wcp_sb = []
                for ci in range(2):
                    wcp_ps = ps_q.tile([P, C], dt.float32, tag="q")
                    for mt in range(2):
                        nc.tensor.matmul(wcp_ps[:], w_atmc_t_sb[mt][:, ci * P:(ci + 1) * P],
                                         w1_sb[mt][:], start=(mt == 0), stop=(mt == 1))
                    wcp = small_pool.tile([P, C], dt.bfloat16, tag=f"wcp{ci}")
                    nc.vector.tensor_copy(wcp[:], wcp_ps[:])
                    wcp_sb.append(wcp)
                btot = []
                for ot in range(2):
                    vv_ps = ps_q.tile([P, 1], dt.float32, tag="q")
                    for ct in range(2):
                        nc.tensor.matmul(vv_ps[:], ws_sb[ct][:, ot * P:(ot + 1) * P],
                                         b_n_h_sb[ct][:], start=(ct == 0), stop=(ct == 1))
                    bt = small_pool.tile([P, 1], dt.float32, tag=f"bt{ot}")
                    nc.vector.scalar_tensor_tensor(bt[:], vv_ps[:], 1.0,
                                                   b_proj_sb[ot][:], OP.mult, OP.add)
                    btot.append(bt)

                if stop_after == "gating":
                    continue
                # ---- final matmuls + bias + store ----
                for ot in range(2):
                    for q in range(NCH):
                        o_ps = ps_s.tile([P, CH], dt.float32, tag="s")
                        for j in range(CH // 512):
                            n0 = q * CH + j * 512
                            psl = slice(j * 512, (j + 1) * 512)
                            for ct in range(2):
                                nc.tensor.matmul(
                                    o_ps[:, psl], ws_sb[ct][:, ot * P:(ot + 1) * P],
                                    s_sb[ct][q // 2][:, (q % 2) * CH + j * 512:
                                                     (q % 2) * CH + (j + 1) * 512],
                                    start=(ct == 0), stop=False)
                            for ci in range(2):
                                nc.tensor.matmul(
                                    o_ps[:, psl], wcp_sb[ci][:, ot * P:(ot + 1) * P],
                                    xb_sb[ci][:, PAD + n0:PAD + n0 + 512],
                                    start=False, stop=(ci == 1))
                        o_sb = osb_pool.tile([P, CH], dt.float32, tag="osb")
                        nc.scalar.activation(o_sb[:], o_ps[:], AF.Identity, bias=btot[ot][:])
                        nc.sync.dma_start(
                            out_d[b, ot * P:(ot + 1) * P, q * CH:(q + 1) * CH], o_sb[:])
    nc.compile()
    return nc


def _host_prep(inputs):
    x = inputs["x"].astype(np.float32)
    ln_g, ln_b = inputs["ln_g"], inputs["ln_b"]
    off_w, off_b = inputs["off_w"], inputs["off_b"]
    w_off = (ln_g[:, None] * off_w).astype(np.float32)
    off_b_eff = ln_b @ off_w + off_b
    assert np.allclose(off_b_eff, 0.0), "offset bias path not implemented"
    neg_colsum = (-w_off.sum(0, dtype=np.float64)).astype(np.float32)[None, :]
    fc2_w, fc2_b = inputs["fc2_w"], inputs["fc2_b"]
    fc2d_w = (fc2_w[:, 0::2] - fc2_w[:, 1::2]).astype(np.float32)
    fc2d_b = (fc2_b[0::2] - fc2_b[1::2]).astype(np.float32)
    xt = np.ascontiguousarray(x.transpose(0, 2, 1))
    xt = np.pad(xt, ((0, 0), (0, 0), (PAD, PAD))).astype(bf16)
    common = {
        "w_off": w_off.astype(bf16),
        "neg_colsum": neg_colsum.astype(bf16),
        "w_n": inputs["atm_n_w"].astype(bf16),
        "w_atmc": inputs["atm_c_w"].astype(bf16),
        "w_atmc_t": np.ascontiguousarray(inputs["atm_c_w"].T).astype(bf16),
        "w_proj": inputs["proj_w"].astype(np.float32),
        "b_proj": inputs["proj_b"].astype(np.float32)[:, None],
        "b_n": inputs["atm_n_b"].astype(np.float32)[:, None],
        "b_n_h": inputs["atm_n_b"].astype(bf16)[:, None],
        "w_fc1": inputs["fc1_w"].astype(bf16),
        "b_fc1": inputs["fc1_b"].astype(np.float32)[:, None],
        "b_fc1s": (inputs["fc1_b"] * 0.7071067811865476).astype(np.float32)[:, None],
        "w_fc2d": fc2d_w.astype(bf16),
        "b_fc2d": fc2d_b[:, None],
    }
    in_maps = []
    for core in range(NCORES):
        m = dict(common)
        m["xb"] = np.ascontiguousarray(xt[core * BLOC:(core + 1) * BLOC])
        in_maps.append(m)
    return in_maps


def kernel(**inputs):
    inputs = {k: np.asarray(v) for k, v in inputs.items()}
    assert np.all(inputs["attn_mask"] == 1), "kernel specialized for all-ones mask"
    in_maps = _host_prep(inputs)
    if "nc" not in _cache:
        _cache["nc"] = _build_program()
    res = None
    for attempt in range(3):
        try:
            res = run_bass_kernel_spmd(_cache["nc"], in_maps, list(range(NCORES)))
            break
        except Exception:
            # transient NRT_EXEC_UNIT_UNRECOVERABLE wedges have been observed
            # on this fabric; back off and retry before giving up
            if attempt == 2:
                raise
            import time
            time.sleep(10)
    outs = [res.results[i]["out_t"] for i in range(NCORES)]
    full = np.concatenate(outs, axis=0)
    return np.ascontiguousarray(full.transpose(0, 2, 1))


if __name__ == "__main__":
    import reference
    inp = {k: np.asarray(v) for k, v in reference.setup_inputs().items()}
    out = kernel(**inp)
    print("kernel out", out.shape, out.dtype)
